# revision 1
# baseline (speedup 1.0000x reference)
"""GCN link predictor on 8 TRN2 NeuronCores (Bass/Tile).

Sharding: dst-node partitioning. Core k owns nodes [5000k, 5000(k+1)).
Each GCNConv layer:
  - per-core dense matmul p = x @ W (own node slice), scaled by dinv[node]
  - AllGather of the scaled feature table (fp16) to every core's HBM
  - per-core gather of p~[src] rows (dma_gather, 256B rows) for the edges
    whose dst lands in the core's range, edges pre-sorted by dst-tile
  - segmented scatter-add via indicator matmuls on the PE: for each chunk
    of 128 edges, Ind[e, d] = (dst_local[e] == d) * dinv[dst_e] built on
    the DVE, then PSUM += msg^T @ Ind (conv1, transposed) / Ind^T @ msg
    (conv2).
Decode: AllGather z, data-parallel pair gathers + DVE multiply-reduce.

dma_gather indices are int16, so each 40000-row table is addressed through
two overlapping views: A = rows [0, 32768), B = rows [7232, 40000). Edges
are assigned to a view per (core, dst-tile) with chunk-count balancing.
"""

import numpy as np

import concourse.bass as bass
import concourse.bacc as bacc
import concourse.mybir as mybir
import concourse.tile as tile
from concourse.bass_utils import run_bass_kernel_spmd

P = 128
N_NODES = 40000
IN_CH = 128
HID_CH = 128
OUT_CH = 64
N_LABEL = 200000
N_CORES = 8
NPC = N_NODES // N_CORES          # 5000 nodes per core
NT = (NPC + P - 1) // P           # 40 dst tiles per core (last has 8 nodes)
A_LIM = 32768                     # view A = rows [0, 32768)
B_OFF = N_NODES - A_LIM           # 7232; view B = rows [7232, 40000)
GROUP = 4                         # dst tiles per gather group
LPC = N_LABEL // N_CORES          # 25000 label pairs per core
LBATCH = 32                       # label chunks per decode gather batch

F16 = mybir.dt.float16
F32 = mybir.dt.float32
I16 = mybir.dt.int16


def _wrap16(flat):
    """Lay out a flat index list in dma_gather's expected SBUF image:
    position n -> [n % 16, n // 16], replicated across the 8 groups of 16
    partitions. Returns [128, len/16] int16."""
    n = len(flat)
    assert n % 16 == 0
    grid = np.asarray(flat, np.int16).reshape(n // 16, 16).T
    return np.tile(grid, (8, 1))


def _prepare(x, edge_index, edge_label_index, W1, b1, W2, b2):
    src = np.asarray(edge_index[0], np.int64)
    dst = np.asarray(edge_index[1], np.int64)
    loops = np.arange(N_NODES, dtype=np.int64)
    fsrc = np.concatenate([src, loops])
    fdst = np.concatenate([dst, loops])
    deg = np.bincount(fdst, minlength=N_NODES).astype(np.float64)
    dinv = (1.0 / np.sqrt(deg)).astype(np.float32)

    # ---- bucket edges by (core, tile), sorted by src within each bucket
    core_of = fdst // NPC
    tloc = (fdst % NPC) // P
    order = np.lexsort((fsrc, tloc, core_of))
    s_src = fsrc[order]
    s_dst = fdst[order]
    s_core = core_of[order]
    s_tile = tloc[order]
    # bucket boundaries
    key = s_core * NT + s_tile
    starts = np.searchsorted(key, np.arange(N_CORES * NT))
    ends = np.searchsorted(key, np.arange(N_CORES * NT) + 1)

    cnt = (ends - starts).reshape(N_CORES, NT)
    # forced-A (< B_OFF) / forced-B (>= A_LIM) counts per bucket
    fA = np.empty((N_CORES, NT), np.int64)
    fB = np.empty((N_CORES, NT), np.int64)
    for k in range(N_CORES):
        for t in range(NT):
            b = k * NT + t
            ss = s_src[starts[b]:ends[b]]
            fA[k, t] = np.searchsorted(ss, B_OFF)
            fB[k, t] = len(ss) - np.searchsorted(ss, A_LIM)
    NCA = np.maximum(1, (fA.max(axis=0) + P - 1) // P)          # [NT]
    nA = np.minimum(cnt - fB, P * NCA[None, :])                 # [cores, NT]
    nA = np.maximum(nA, 0)
    cntB = cnt - nA
    NCB = (cntB.max(axis=0) + P - 1) // P                       # [NT]
    NCT = NCA + NCB

    groups = []
    tile_chunks = {}
    gbase = 0
    col = 0
    for g0 in range(0, NT, GROUP):
        ts = list(range(g0, min(g0 + GROUP, NT)))
        gnA = int(NCA[ts].sum())
        gnB = int(NCB[ts].sum())
        groups.append(dict(tiles=ts, base=gbase, nA=gnA, nB=gnB,
                           colA=col, colB=col + gnA * 8))
        ca = gbase
        cb = gbase + gnA
        for t in ts:
            tile_chunks[t] = (list(range(ca, ca + int(NCA[t])))
                              + list(range(cb, cb + int(NCB[t]))))
            ca += int(NCA[t])
            cb += int(NCB[t])
        gbase += gnA + gnB
        col += (gnA + gnB) * 8
    TOT_CH = gbase
    WC = col

    # ---- per-core conv arrays
    cores = []
    for k in range(N_CORES):
        eidx = np.zeros((P, WC), np.int16)
        edloc = np.zeros((P, TOT_CH), np.float32)
        edinv = np.zeros((P, TOT_CH), np.float32)
        for g in groups:
            flatA = []
            flatB = []
            for t in g["tiles"]:
                b = k * NT + t
                ss = s_src[starts[b]:ends[b]]
                dd = s_dst[starts[b]:ends[b]]
                na = int(nA[k, t])
                la = np.zeros(int(NCA[t]) * P, np.int64)
                wa = np.zeros(int(NCA[t]) * P, np.float32)
                ia = np.zeros(int(NCA[t]) * P, np.int64)
                ia[:na] = ss[:na]
                la[:na] = dd[:na] - k * NPC - t * P
                wa[:na] = dinv[dd[:na]]
                lb_ = np.zeros(int(NCB[t]) * P, np.int64)
                wb = np.zeros(int(NCB[t]) * P, np.float32)
                ib = np.zeros(int(NCB[t]) * P, np.int64)
                nb = int(cntB[k, t])
                ib[:nb] = ss[na:na + nb] - B_OFF
                lb_[:nb] = dd[na:na + nb] - k * NPC - t * P
                wb[:nb] = dinv[dd[na:na + nb]]
                flatA.append((ia, la, wa))
                flatB.append((ib, lb_, wb))
            ia = np.concatenate([f[0] for f in flatA])
            ib = np.concatenate([f[0] for f in flatB])
            locs = np.concatenate([f[1] for f in flatA] + [f[1] for f in flatB])
            ws = np.concatenate([f[2] for f in flatA] + [f[2] for f in flatB])
            if len(ia):
                eidx[:, g["colA"]:g["colA"] + len(ia) // 16] = _wrap16(ia)
            if len(ib):
                eidx[:, g["colB"]:g["colB"] + len(ib) // 16] = _wrap16(ib)
            nch = g["nA"] + g["nB"]
            edloc[:, g["base"]:g["base"] + nch] = \
                locs.reshape(nch, P).T.astype(np.float32)
            edinv[:, g["base"]:g["base"] + nch] = ws.reshape(nch, P).T
        cores.append(dict(eidx=eidx, edloc=edloc, edinv=edinv))

    # ---- label prep: 4 groups by (a_view, b_view) = AA, AB, BA, BB
    la_all = np.asarray(edge_label_index[0], np.int64)
    lb_all = np.asarray(edge_label_index[1], np.int64)
    lab = []
    for k in range(N_CORES):
        a = la_all[k * LPC:(k + 1) * LPC]
        b = lb_all[k * LPC:(k + 1) * LPC]
        gid = (a >= A_LIM).astype(np.int64) * 2 + (b >= A_LIM).astype(np.int64)
        o = np.argsort(gid, kind="stable")
        lab.append((a[o], b[o], gid[o], o))
    lcnt = np.array([[int((lab[k][2] == gi).sum()) for gi in range(4)]
                     for k in range(N_CORES)])
    LC = (lcnt.max(axis=0) + P - 1) // P                         # [4]
    LCH = int(LC.sum())

    # label batches: sub-ranges of one group, LBATCH chunks each
    lbatches = []
    cbase = 0
    acol = [0, 0]
    a_col_base = {0: 0, 1: int((LC[0] + LC[1]) * 8)}  # view A ops, view B op
    # a-op for view A covers groups 0,1; view B covers groups 2,3
    # b-ops: one per group
    b_col_base = {}
    bc = int((LC[0] + LC[1]) * 8 + (LC[2] + LC[3]) * 8)
    for gi in range(4):
        b_col_base[gi] = bc
        bc += int(LC[gi]) * 8
    WL = bc
    a_off_in_op = {0: 0, 1: int(LC[0]), 2: 0, 3: int(LC[2])}
    for gi in range(4):
        av = 0 if gi < 2 else 1
        bv = gi % 2
        for c0 in range(0, int(LC[gi]), LBATCH):
            nch = min(LBATCH, int(LC[gi]) - c0)
            lbatches.append(dict(
                nch=nch, base=cbase + c0,
                a_view=av, b_view=bv,
                a_col=a_col_base[av] + (a_off_in_op[gi] + c0) * 8,
                b_col=b_col_base[gi] + c0 * 8))
        cbase += int(LC[gi])

    perms = []
    for k in range(N_CORES):
        a, b, gid, o = lab[k]
        lidx = np.zeros((P, WL), np.int16)
        perm = np.full(LCH * P, -1, np.int64)
        pos = 0
        cbase = 0
        aA_parts, aB_parts = [], []
        for gi in range(4):
            m = gid == gi
            ga, gb, go = a[m], b[m], o[m]
            npad = int(LC[gi]) * P - len(ga)
            ga = np.concatenate([ga, np.zeros(npad, np.int64)
                                 + (0 if gi < 2 else B_OFF)])
            gb = np.concatenate([gb, np.zeros(npad, np.int64)
                                 + (0 if gi % 2 == 0 else B_OFF)])
            perm[cbase * P: cbase * P + len(go)] = go
            cbase += int(LC[gi])
            av = ga - (0 if gi < 2 else B_OFF)
            bvx = gb - (0 if gi % 2 == 0 else B_OFF)
            if gi < 2:
                aA_parts.append(av)
            else:
                aB_parts.append(av)
            if len(bvx):
                lidx[:, b_col_base[gi]:b_col_base[gi] + len(bvx) // 16] = \
                    _wrap16(bvx)
        aA = np.concatenate(aA_parts) if aA_parts else np.zeros(0, np.int64)
        aB = np.concatenate(aB_parts) if aB_parts else np.zeros(0, np.int64)
        if len(aA):
            lidx[:, 0:len(aA) // 16] = _wrap16(aA)
        if len(aB):
            lidx[:, a_col_base[1]:a_col_base[1] + len(aB) // 16] = _wrap16(aB)
        cores[k]["lidx"] = lidx
        perms.append(perm)

    # ---- dense inputs per core
    for k in range(N_CORES):
        xk = np.asarray(x[k * NPC:(k + 1) * NPC], np.float32)
        cores[k]["xT"] = np.ascontiguousarray(xk.T).astype(np.float16)
        cores[k]["W1h"] = np.asarray(W1, np.float32).astype(np.float16)
        cores[k]["W2h"] = np.asarray(W2, np.float32).astype(np.float16)
        cores[k]["b1col"] = np.asarray(b1, np.float32).reshape(HID_CH, 1)
        cores[k]["b2row"] = np.asarray(b2, np.float32).astype(
            np.float16).reshape(1, OUT_CH)
        dk = np.ones((P, NT), np.float32)
        dv = dinv[k * NPC:(k + 1) * NPC]
        for t in range(NT):
            m = min(P, NPC - t * P)
            dk[:m, t] = dv[t * P:t * P + m]
        cores[k]["dinvk"] = dk

    meta = dict(groups=groups, tile_chunks=tile_chunks, TOT_CH=TOT_CH, WC=WC,
                NCT=[int(v) for v in NCT], lbatches=lbatches, LCH=LCH, WL=WL)
    return meta, cores, perms


DEBUG_STAGE = 4  # 1=p+AGp, 2=+conv1+AGq, 3=+conv2+AGz, 4=+decode (full)
DEBUG_DECODE = "full"  # "full" | "nottr" | "nogather"


def _build(meta):
    TOT_CH, WC, LCH, WL = (meta["TOT_CH"], meta["WC"],
                           meta["LCH"], meta["WL"])
    NCHG_MAX = max(g["nA"] + g["nB"] for g in meta["groups"])

    nc = bacc.Bacc("TRN2", target_bir_lowering=False, debug=False,
                   num_devices=N_CORES)
    xT = nc.dram_tensor("xT", [P, NPC], F16, kind="ExternalInput")
    W1h = nc.dram_tensor("W1h", [P, HID_CH], F16, kind="ExternalInput")
    W2h = nc.dram_tensor("W2h", [P, OUT_CH], F16, kind="ExternalInput")
    b1col = nc.dram_tensor("b1col", [P, 1], F32, kind="ExternalInput")
    b2row = nc.dram_tensor("b2row", [1, OUT_CH], F16, kind="ExternalInput")
    dinvk = nc.dram_tensor("dinvk", [P, NT], F32, kind="ExternalInput")
    eidx = nc.dram_tensor("eidx", [P, WC], I16, kind="ExternalInput")
    edloc = nc.dram_tensor("edloc", [P, TOT_CH], F32, kind="ExternalInput")
    edinv = nc.dram_tensor("edinv", [P, TOT_CH], F32, kind="ExternalInput")
    lidx = nc.dram_tensor("lidx", [P, WL], I16, kind="ExternalInput")
    logits = nc.dram_tensor("logits", [P, LCH], F32, kind="ExternalOutput")

    RG = [list(range(N_CORES))]

    with tile.TileContext(nc) as tc:
        with tc.tile_pool(name="const", bufs=1) as cpool, \
             tc.tile_pool(name="msgp", bufs=2) as msgp, \
             tc.tile_pool(name="indp", bufs=4) as indp, \
             tc.tile_pool(name="evac", bufs=3) as evac, \
             tc.tile_pool(name="decp", bufs=2) as decp, \
             tc.tile_pool(name="psA", bufs=2, space="PSUM") as psA, \
             tc.tile_pool(name="psB", bufs=2, space="PSUM") as psB, \
             tc.tile_pool(name="dram", bufs=1, space="DRAM") as dram:

            # constants into SBUF
            xT_s = cpool.tile([P, NPC], F16)
            W1_s = cpool.tile([P, HID_CH], F16)
            W2_s = cpool.tile([P, OUT_CH], F16)
            b1_s = cpool.tile([P, 1], F32)
            b2_s = cpool.tile([1, OUT_CH], F16)
            dk_s = cpool.tile([P, NT], F32)
            ei_s = cpool.tile([P, WC], I16)
            el_s = cpool.tile([P, TOT_CH], F32)
            ew_s = cpool.tile([P, TOT_CH], F32)
            li_s = cpool.tile([P, WL], I16)
            ones_s = cpool.tile([1, P], F16)
            iota_s = cpool.tile([P, P], F16)
            nc.sync.dma_start(out=xT_s[:], in_=xT[:])
            nc.sync.dma_start(out=W1_s[:], in_=W1h[:])
            nc.sync.dma_start(out=W2_s[:], in_=W2h[:])
            nc.sync.dma_start(out=b1_s[:], in_=b1col[:])
            nc.sync.dma_start(out=b2_s[:], in_=b2row[:])
            nc.sync.dma_start(out=dk_s[:], in_=dinvk[:])
            nc.sync.dma_start(out=ei_s[:], in_=eidx[:])
            nc.sync.dma_start(out=el_s[:], in_=edloc[:])
            nc.sync.dma_start(out=ew_s[:], in_=edinv[:])
            nc.sync.dma_start(out=li_s[:], in_=lidx[:])
            nc.vector.memset(ones_s[:], 1.0)
            nc.gpsimd.iota(iota_s[:], pattern=[[1, P]], base=0,
                           channel_multiplier=0,
                           allow_small_or_imprecise_dtypes=True)

            p_in = dram.tile([NPC, HID_CH], F16)
            PT = dram.tile([N_NODES, HID_CH], F16, addr_space="Shared")
            q_in = dram.tile([NPC, P], F16)
            QT = dram.tile([N_NODES, P], F16, addr_space="Shared")
            z_in = dram.tile([NPC, P], F16)
            ZT = dram.tile([N_NODES, P], F16, addr_space="Shared")

            # ---- stage 1: p~ = (x @ W1) * dinv[node]
            for t in range(NT):
                m = min(P, NPC - t * P)
                psum_p = psB.tile([P, HID_CH], F32, tag="pp", space="PSUM")
                nc.tensor.matmul(out=psum_p[0:m, :],
                                 lhsT=xT_s[:, t * P:t * P + m],
                                 rhs=W1_s[:], start=True, stop=True)
                p_sb = evac.tile([P, HID_CH], F16, tag="pev")
                nc.scalar.activation(out=p_sb[0:m, :], in_=psum_p[0:m, :],
                                     func=mybir.ActivationFunctionType.Copy,
                                     scale=dk_s[0:m, t:t + 1])
                nc.sync.dma_start(out=p_in[t * P:t * P + m, :],
                                  in_=p_sb[0:m, :])

            nc.gpsimd.collective_compute(
                "AllGather", mybir.AluOpType.bypass, replica_groups=RG,
                ins=[p_in.opt()], outs=[PT.opt()])

            # ---- stage 2: conv1 aggregation (transposed) + q~
            def conv_layer(TBL, out_dram, is_conv1):
                for g in meta["groups"]:
                    nch = g["nA"] + g["nB"]
                    msg = msgp.tile([P, NCHG_MAX, P], F16, tag="msg")
                    if g["nA"]:
                        nc.gpsimd.dma_gather(
                            out_ap=msg[:, 0:g["nA"], :],
                            in_ap=TBL[0:A_LIM, :],
                            idxs_ap=ei_s[:, g["colA"]:g["colA"] + g["nA"] * 8],
                            num_idxs=g["nA"] * P, num_idxs_reg=g["nA"] * P,
                            elem_size=P, single_packet=False)
                    if g["nB"]:
                        nc.gpsimd.dma_gather(
                            out_ap=msg[:, g["nA"]:nch, :],
                            in_ap=TBL[B_OFF:N_NODES, :],
                            idxs_ap=ei_s[:, g["colB"]:g["colB"] + g["nB"] * 8],
                            num_idxs=g["nB"] * P, num_idxs_reg=g["nB"] * P,
                            elem_size=P, single_packet=False)
                    for t in g["tiles"]:
                        m = min(P, NPC - t * P)
                        chunks = meta["tile_chunks"][t]
                        if is_conv1:
                            ps = psA.tile([HID_CH, P], F32, tag="agg1",
                                          space="PSUM")
                        else:
                            ps = psA.tile([P, OUT_CH], F32, tag="agg2",
                                          space="PSUM")
                            nc.tensor.matmul(out=ps[:], lhsT=ones_s[:],
                                             rhs=b2_s[:], start=True,
                                             stop=False)
                        for ci, gc in enumerate(chunks):
                            lc = gc - g["base"]
                            ind = indp.tile([P, P], F16, tag="ind")
                            nc.vector.tensor_scalar(
                                out=ind[:], in0=iota_s[:],
                                scalar1=el_s[:, gc:gc + 1],
                                scalar2=ew_s[:, gc:gc + 1],
                                op0=mybir.AluOpType.is_equal,
                                op1=mybir.AluOpType.mult)
                            if is_conv1:
                                nc.tensor.matmul(
                                    out=ps[:], lhsT=msg[:, lc, :], rhs=ind[:],
                                    start=(ci == 0),
                                    stop=(ci == len(chunks) - 1))
                            else:
                                nc.tensor.matmul(
                                    out=ps[:], lhsT=ind[:],
                                    rhs=msg[:, lc, 0:OUT_CH],
                                    start=False,
                                    stop=(ci == len(chunks) - 1))
                        if is_conv1:
                            hT = evac.tile([HID_CH, P], F16, tag="hT")
                            nc.scalar.activation(
                                out=hT[:], in_=ps[:],
                                func=mybir.ActivationFunctionType.Relu,
                                bias=b1_s[:, 0:1])
                            psq = psB.tile([P, OUT_CH], F32, tag="pq",
                                           space="PSUM")
                            nc.tensor.matmul(out=psq[0:m, :],
                                             lhsT=hT[:, 0:m], rhs=W2_s[:],
                                             start=True, stop=True)
                            qsb = evac.tile([P, P], F16, tag="qev")
                            nc.vector.memset(qsb[:, OUT_CH:P], 0)
                            nc.scalar.activation(
                                out=qsb[0:m, 0:OUT_CH], in_=psq[0:m, :],
                                func=mybir.ActivationFunctionType.Copy,
                                scale=dk_s[0:m, t:t + 1])
                            nc.sync.dma_start(
                                out=out_dram[t * P:t * P + m, :],
                                in_=qsb[0:m, :])
                        else:
                            zsb = evac.tile([P, P], F16, tag="qev")
                            nc.vector.memset(zsb[:, OUT_CH:P], 0)
                            nc.scalar.copy(out=zsb[0:m, 0:OUT_CH],
                                           in_=ps[0:m, :])
                            nc.sync.dma_start(
                                out=out_dram[t * P:t * P + m, :],
                                in_=zsb[0:m, :])

            if DEBUG_STAGE >= 2:
                conv_layer(PT, q_in, True)
                nc.gpsimd.collective_compute(
                    "AllGather", mybir.AluOpType.bypass, replica_groups=RG,
                    ins=[q_in.opt()], outs=[QT.opt()])
            if DEBUG_STAGE >= 3:
                conv_layer(QT, z_in, False)
                nc.gpsimd.collective_compute(
                    "AllGather", mybir.AluOpType.bypass, replica_groups=RG,
                    ins=[z_in.opt()], outs=[ZT.opt()])

            # ---- decode
            logit_sb = cpool.tile([P, LCH], F32)
            nc.vector.memset(logit_sb[:], 0)
            for b in (meta["lbatches"] if DEBUG_STAGE >= 4 else []):
                nch = b["nch"]
                za = decp.tile([P, LBATCH, P], F16, tag="za")
                zb = decp.tile([P, LBATCH, P], F16, tag="zb")
                av = (0, A_LIM) if b["a_view"] == 0 else (B_OFF, N_NODES)
                bv = (0, A_LIM) if b["b_view"] == 0 else (B_OFF, N_NODES)
                if DEBUG_DECODE != "nogather":
                    nc.gpsimd.dma_gather(
                        out_ap=za[:, 0:nch, :], in_ap=ZT[av[0]:av[1], :],
                        idxs_ap=li_s[:, b["a_col"]:b["a_col"] + nch * 8],
                        num_idxs=nch * P, num_idxs_reg=nch * P,
                        elem_size=P, single_packet=False)
                    nc.gpsimd.dma_gather(
                        out_ap=zb[:, 0:nch, :], in_ap=ZT[bv[0]:bv[1], :],
                        idxs_ap=li_s[:, b["b_col"]:b["b_col"] + nch * 8],
                        num_idxs=nch * P, num_idxs_reg=nch * P,
                        elem_size=P, single_packet=False)
                else:
                    nc.vector.memset(za[:], 0)
                    nc.vector.memset(zb[:], 0)
                if DEBUG_DECODE == "gatheronly":
                    continue
                # ACT firewall: DVE reading dma_gather-written SBUF directly
                # wedges the device in the full program; route through ACT.
                za2 = decp.tile([P, LBATCH, OUT_CH], F16, tag="za2")
                zb2 = decp.tile([P, LBATCH, OUT_CH], F16, tag="zb2")
                nc.scalar.copy(out=za2[:, 0:nch, :], in_=za[:, 0:nch, 0:OUT_CH])
                nc.scalar.copy(out=zb2[:, 0:nch, :], in_=zb[:, 0:nch, 0:OUT_CH])
                scr = decp.tile([P, LBATCH, OUT_CH], F16, tag="dscr")
                nc.vector.tensor_tensor(
                    out=scr[:, 0:nch, :], in0=za2[:, 0:nch, :],
                    in1=zb2[:, 0:nch, :], op=mybir.AluOpType.mult)
                nc.vector.tensor_reduce(
                    out=logit_sb[:, b["base"]:b["base"] + nch],
                    in_=scr[:, 0:nch, :], axis=mybir.AxisListType.X,
                    op=mybir.AluOpType.add)
            nc.sync.dma_start(out=logits[:], in_=logit_sb[:])

    nc.compile()
    return nc


_CACHE = {}
TRACE = False          # set True (e.g. from test.py) to capture an NTFF trace
LAST_RESULT = None     # BassKernelResults of the most recent run


def kernel(**inputs):
    meta, cores, perms = _prepare(**inputs)
    key = (meta["TOT_CH"], meta["LCH"], meta["WC"], meta["WL"],
           tuple(meta["NCT"]))
    if key not in _CACHE:
        _CACHE[key] = _build(meta)
    nc = _CACHE[key]
    in_maps = [dict(xT=c["xT"], W1h=c["W1h"], W2h=c["W2h"], b1col=c["b1col"],
                    b2row=c["b2row"], dinvk=c["dinvk"], eidx=c["eidx"],
                    edloc=c["edloc"], edinv=c["edinv"], lidx=c["lidx"])
               for c in cores]
    res = run_bass_kernel_spmd(nc, in_maps, core_ids=list(range(N_CORES)),
                               trace=TRACE)
    global LAST_RESULT
    LAST_RESULT = res
    out = np.empty(N_LABEL, np.float32)
    for k in range(N_CORES):
        vals = res.results[k]["logits"].T.ravel()
        perm = perms[k]
        m = perm >= 0
        out[k * LPC + perm[m]] = vals[m]
    return out

